# revision 24
# baseline (speedup 1.0000x reference)
"""BiLSTM+CRF loss kernel for Trainium2 (8 NeuronCores, data-parallel over batch).

Model (B=128, T=512, V=30000, E=100, H=128/dir, K=9 tags):
  embeds = embedding[x]; bi-LSTM over T; emissions = FC(h_cat); loss = -mean(CRF llh).

Sharding: batch 128 -> 16 sequences per core (data parallel, params replicated).
Each core returns llh[16]; host sums and negates -> scalar loss.

Device pipeline per core:
  1. Batched indirect-DMA gather of embeddings (8 calls x 8 token-tiles, korder
     so both LSTM ends' chunks land first), PE-transpose -> embT [E+1, TOK] bf16.
  2. Input projections (xp) per 16-step chunk into PSUM (gate-major); next
     chunk's xp matmuls interleaved into the step loop (keeps PE warm).
     Per step: 8 tiny recurrence matmuls accumulate W_hh @ h; one fused Sigmoid
     over all gates/dirs (tanh gate via tanh(x) = 2*sigmoid(2x)-1, weights
     pre-doubled); cell update fused via scalar_tensor_tensor with a c/2-scaled
     cell state and tanh(scale=2); h written transposed (ready as next rhs).
     Fwd and bwd LSTM run concurrently (fwd t ascending, bwd t descending).
  3. FC -> emissions [tok, 9]; gold-path score (num) via one-hot DVE bulk ops.
  4. CRF partition function via a *product tree*: Z_b = a0^T G_1 G_2..G_511 q
     with G_t = exp(trans) * diag-col exp(e_t).  8 time-blocks of 64 per
     sequence live on one partition each (p = blk*16 + b); 6 levels of pairwise
     9x9 matrix products (bulk DVE ops, 4-free-dim APs), sum-normalized per
     level with ln-corrections summed at the end; tiny 8-step combine chain.

mask is all-ones per the problem spec (fill: ones) and is not applied on device.
"""

import functools

import numpy as np
from contextlib import ExitStack

import concourse.bass as bass
import concourse.bacc as bacc
import concourse.hw_specs as hw_specs
import concourse.mybir as mybir
import concourse.tile as tile
from concourse.masks import make_identity

dt = mybir.dt
F32 = dt.float32
BF16 = dt.bfloat16
I32 = dt.int32
ALU = mybir.AluOpType
ACTF = mybir.ActivationFunctionType
AXL = mybir.AxisListType

BL = 16          # sequences per core
E = 100          # embedding dim
H = 128          # hidden per direction
K = 9            # tags
G = 4            # gates
TPC = 8          # timesteps per xp chunk (128 tokens = one token-tile)
NBG = 8          # token-tiles per gather batch
DIRSPLIT = False  # split fwd/bwd LSTM chains into independent instruction streams


_orig_act_tables = hw_specs.get_activation_tables


@functools.cache
def _pinned_act_tables(arch):
    """Pin Sigmoid/Tanh to one table set and Exp/Ln to another so the
    act-table chooser never alternates sets inside the hot loops
    (each InstLoadActFuncSet costs ~1.3us on the Scalar engine)."""
    AF = mybir.ActivationFunctionType
    tabs = {k: set(v) for k, v in _orig_act_tables(arch).items()}
    keep = {AF.Sigmoid: "sigmoid_and_others", AF.Tanh: "sigmoid_and_others",
            AF.Exp: "natural_log_exp_and_others", AF.Ln: "natural_log_exp_and_others"}
    for fn, home in keep.items():
        assert fn in tabs[home], (fn, home)
        for name, fs in tabs.items():
            if name != home:
                fs.discard(fn)
    return tabs


hw_specs.get_activation_tables = _pinned_act_tables
bacc.get_activation_tables = _pinned_act_tables


def _mm(ap):
    """matmul operand view: f32 storage computes as f32r (full-rate, TF32-ish)."""
    return ap.bitcast(dt.float32r) if ap.dtype == F32 else ap


def _ap(base, extra_off, dims):
    """Manual AP: same tensor as `base`, base.offset + extra_off, given [step,count] dims."""
    return bass.AP(base.tensor, base.offset + extra_off, dims)


def _korder(ntile):
    ko = []
    lo, hi = 0, ntile - 1
    while lo <= hi:
        ko.append(lo)
        if hi != lo:
            ko.append(hi)
        lo += 1
        hi -= 1
    return ko


def build_program(T=512, V=30000, wbf=False, hbf=False):
    WDT = BF16 if wbf else F32   # weight storage (wih/whh/fct)
    HDT = BF16 if hbf else F32   # activation storage (embT/hist)
    TOK = T * BL
    NTILE = TOK // 128        # 128-token tiles
    NCH = T // TPC            # xp chunks
    CHTOK = TPC * BL          # tokens per chunk = 256
    HB = 8 * H                # 1024: (dir,gate) blocks of H cols
    NBLK = 8                  # CRF time blocks per sequence
    UB = T // NBLK            # timesteps per block = 64
    assert UB == 64 and NBLK * BL == 128
    ESW = UB * K              # estream row width = 576

    nc = bacc.Bacc(None, target_bir_lowering=False, debug=False)

    # ---------------- DRAM I/O ----------------
    idx_d = nc.dram_tensor("idx", [128, TOK // 128], I32, kind="ExternalInput")  # korder-permuted, p-major
    tga_d = nc.dram_tensor("tga", [128, TOK // 128], F32, kind="ExternalInput")
    tgb_d = nc.dram_tensor("tgb", [128, TOK // 128], F32, kind="ExternalInput")
    emb_d = nc.dram_tensor("emb", [V, E], F32, kind="ExternalInput")
    wih_d = nc.dram_tensor("wih", [E + 1, HB], WDT, kind="ExternalInput")
    whh_d = nc.dram_tensor("whh", [H, HB], WDT, kind="ExternalInput")
    fct_d = nc.dram_tensor("fct", [H, 2 * K], WDT, kind="ExternalInput")
    fcb_d = nc.dram_tensor("fcb", [128, K], F32, kind="ExternalInput")
    iot_d = nc.dram_tensor("iot", [128, K], F32, kind="ExternalInput")
    t81_d = nc.dram_tensor("t81", [128, K * K], F32, kind="ExternalInput")
    tex_d = nc.dram_tensor("tex", [128, K * K], F32, kind="ExternalInput")
    i81_d = nc.dram_tensor("i81", [BL, K * K], F32, kind="ExternalInput")
    sxp_d = nc.dram_tensor("sxp", [BL, K], F32, kind="ExternalInput")
    exq_d = nc.dram_tensor("exq", [BL, K], F32, kind="ExternalInput")
    srp_d = nc.dram_tensor("srp", [BL, K], F32, kind="ExternalInput")
    erp_d = nc.dram_tensor("erp", [BL, K], F32, kind="ExternalInput")
    tg0_d = nc.dram_tensor("tg0", [BL, 1], F32, kind="ExternalInput")
    tgL_d = nc.dram_tensor("tgL", [BL, 1], F32, kind="ExternalInput")
    one_d = nc.dram_tensor("one", [1, TOK], HDT, kind="ExternalInput")
    llh_d = nc.dram_tensor("llh", [BL, 1], F32, kind="ExternalOutput")

    korder = _korder(NTILE)

    with tile.TileContext(nc) as tc, ExitStack() as ctx:
        const = ctx.enter_context(tc.tile_pool(name="const", bufs=1))
        pers = ctx.enter_context(tc.tile_pool(name="pers", bufs=1))

        # ---- persistent SBUF ----
        NSEG = min(4, TOK // CHTOK)
        WSEG = TOK // NSEG
        assert WSEG % CHTOK == 0
        embT = [pers.tile([128, WSEG], HDT, name=f"embT{i}", tag=f"embT{i}")
                for i in range(NSEG)]
        hist = pers.tile([128, 2 * TOK], HDT)      # h^T history: fwd cols [0,TOK), bwd +TOK
        emsb = pers.tile([128, NTILE * K], F32)     # emissions, tok-partition layout
        emcol = pers.tile([128, NTILE], F32)
        trcol = pers.tile([128, NTILE], F32)
        estr = pers.tile([128, ESW], F32)           # exp(e): p=(blk,b), col=(t%64)*9+j

        wih_s = const.tile([128, HB], WDT)
        whh_s = const.tile([128, HB], WDT)
        fct_s = const.tile([128, 2 * K], WDT)
        fcb_s = const.tile([128, K], F32)
        iot_s = const.tile([128, K], F32)
        t81_s = const.tile([128, K * K], F32)
        tex_s = const.tile([128, K * K], F32)
        i81_s = const.tile([BL, K * K], F32)
        sxp_s = const.tile([BL, K], F32)
        exq_s = const.tile([BL, K], F32)
        srp_s = const.tile([BL, K], F32)
        erp_s = const.tile([BL, K], F32)
        tg0_s = const.tile([BL, 1], F32)
        tgL_s = const.tile([BL, 1], F32)
        ident = const.tile([128, 128], F32)
        idx_s = const.tile([128, NTILE], I32)
        tga_s = const.tile([128, NTILE], F32)
        tgb_s = const.tile([128, NTILE], F32)

        # LSTM state: c' = c/2 (tanh read uses scale=2)
        c_t = pers.tile([128, 2 * BL], F32)

        # ---- const loads ----
        # idx first (gates the gather), then LSTM weights, on the sync queue;
        # tail-phase consts go to the scalar/vector queues so ~20 serialized
        # DGE setups (~1.2us each) don't delay the gather start.
        nc.sync.dma_start(out=idx_s[:], in_=idx_d[:])
        nc.sync.dma_start(out=wih_s[0:E + 1, :], in_=wih_d[:])
        nc.sync.dma_start(out=whh_s[0:H, :], in_=whh_d[:])
        for sg in range(NSEG):
            nc.sync.dma_start(out=embT[sg][E:E + 1, :],
                              in_=one_d[0:1, sg * WSEG:(sg + 1) * WSEG])
        nc.scalar.dma_start(out=fct_s[0:H, :], in_=fct_d[:])
        nc.scalar.dma_start(out=fcb_s[:], in_=fcb_d[:])
        nc.scalar.dma_start(out=iot_s[:], in_=iot_d[:])
        nc.scalar.dma_start(out=t81_s[:], in_=t81_d[:])
        nc.scalar.dma_start(out=tex_s[:], in_=tex_d[:])
        nc.scalar.dma_start(out=i81_s[:], in_=i81_d[:])
        nc.scalar.dma_start(out=sxp_s[:], in_=sxp_d[:])
        nc.scalar.dma_start(out=exq_s[:], in_=exq_d[:])
        nc.scalar.dma_start(out=srp_s[:], in_=srp_d[:])
        nc.scalar.dma_start(out=erp_s[:], in_=erp_d[:])
        nc.scalar.dma_start(out=tg0_s[:], in_=tg0_d[:])
        nc.scalar.dma_start(out=tgL_s[:], in_=tgL_d[:])
        nc.scalar.dma_start(out=tga_s[:], in_=tga_d[:])
        nc.scalar.dma_start(out=tgb_s[:], in_=tgb_d[:])
        make_identity(nc, ident[:])
        nc.vector.memset(c_t[:], 0.0)

        # ---- phase 1: batched gather + transpose -> embT ----
        with tc.tile_pool(name="gath", bufs=3) as gp, \
             tc.tile_pool(name="tpp", bufs=2, space="PSUM") as tpp:
            for g0 in range(0, NTILE, NBG):
                gt = gp.tile([128, NBG * E], F32)
                nc.gpsimd.indirect_dma_start(
                    out=gt[:], out_offset=None, in_=emb_d[:],
                    in_offset=bass.IndirectOffsetOnAxis(
                        ap=idx_s[:, g0:g0 + NBG], axis=0))
                for j in range(NBG):
                    k = korder[g0 + j]
                    pt = tpp.tile([128, 128], F32)
                    nc.tensor.transpose(out=pt[0:E, :], in_=gt[:, j * E:(j + 1) * E],
                                        identity=ident[:])
                    sg, sc = (k * 128) // WSEG, (k * 128) % WSEG
                    if j % 2 == 0:
                        nc.vector.tensor_copy(out=embT[sg][0:E, sc:sc + 128],
                                              in_=pt[0:E, :])
                    else:
                        nc.scalar.activation(out=embT[sg][0:E, sc:sc + 128],
                                             in_=pt[0:E, :], func=ACTF.Copy)

        # ---- phase 2: xp chunks + recurrence (+ in-loop FC / e_scr DMA) ----
        # gates psum layout (slot-major): col = slot*128 + d*64 + g*16 + b
        scr = ctx.enter_context(tc.tile_pool(name="scr", bufs=1, space="DRAM"))
        e_scr = scr.tile([TOK, K], F32)   # flat; addr = b*T*9 + t*9 + j
        TPT = 128 // BL                    # timesteps per token-tile = 8
        GW = TPC * 128                     # gates tile width = 1024

        def emit_xp(g_ap, ch, slot):
            """Emit xp matmul #slot (slot 0..7 -> (d, g)) for chunk ch.
            gates psum layout: col = d*512 + g*128 + (t%TPC)*16 + b."""
            d, g = slot // G, slot % G
            cc = ch if d == 0 else NCH - 1 - ch
            sg, sc = (cc * CHTOK) // WSEG, (cc * CHTOK) % WSEG
            rhs = embT[sg][0:E + 1, sc:sc + CHTOK]
            nc.tensor.matmul(
                g_ap[:, d * 512 + g * 128:d * 512 + (g + 1) * 128],
                _mm(wih_s[0:E + 1, (d * G + g) * H:(d * G + g + 1) * H]),
                _mm(rhs), start=(g == 0), stop=False,
                skip_group_check=True)

        def emit_step(s, g_ap, lsp):
            tf, tb = s, T - 1 - s
            cf, cb = (tf % TPC) * BL, 512 + (tb % TPC) * BL
            if s > 0:
                for d, t, col in ((0, tf, cf), (1, tb, cb)):
                    pcol = (t - 1) * BL if d == 0 else (t + 1) * BL
                    rhs = hist[:, d * TOK + pcol: d * TOK + pcol + BL]
                    for g in range(G):
                        nc.tensor.matmul(
                            _ap(g_ap, col + g * 128, [[GW, 128], [1, BL]]),
                            _mm(whh_s[0:H, (d * G + g) * H:(d * G + g + 1) * H]),
                            _mm(rhs), start=False, stop=True, skip_group_check=True)
            sig = lsp.tile([128, 128], F32, tag="sig")
            up = lsp.tile([128, 2 * BL], F32, tag="up")
            ax = lsp.tile([128, 2 * BL], F32, tag="ax")
            thc = lsp.tile([128, 2 * BL], F32, tag="thc")
            # sigmoid over all (d, g, b); sig layout col = d*64 + g*16 + b
            nc.scalar.activation(
                out=sig[:],
                in_=_ap(g_ap, cf, [[GW, 128], [cb - cf, 2], [128, 4], [1, BL]]),
                func=ACTF.Sigmoid)
            # sig layout: col = d*64 + g*16 + b, gates (i,f,g,o)
            dgb = [[128, 128], [64, 2], [1, BL]]
            # u' = (sig_g - 0.5) * sig_i   [= sig_i * tanh(g) / 2]
            nc.vector.scalar_tensor_tensor(
                out=up[:], in0=_ap(sig[:], 2 * BL, dgb), scalar=-0.5,
                in1=_ap(sig[:], 0, dgb), op0=ALU.add, op1=ALU.mult)
            # a = sig_f * c'
            nc.vector.tensor_tensor(
                out=ax[:], in0=_ap(sig[:], BL, dgb), in1=c_t[:], op=ALU.mult)
            nc.vector.tensor_tensor(out=c_t[:], in0=ax[:], in1=up[:], op=ALU.add)
            nc.scalar.activation(out=thc[:], in_=c_t[:], func=ACTF.Tanh, scale=2.0)
            # h = sig_o * tanh(c) -> hist (both dirs, strided)
            hstep = TOK + (tb - tf) * BL
            nc.vector.tensor_tensor(
                out=_ap(hist[:], tf * BL, [[2 * TOK, 128], [hstep, 2], [1, BL]]),
                in0=_ap(sig[:], 3 * BL, dgb), in1=thc[:], op=ALU.mult)

        def emit_fc(k, fcp):
            """FC + bias for token-tile k -> emsb, then bounce to e_scr."""
            pe = fcp.tile([128, K], F32, tag="pe")
            nc.tensor.matmul(pe[:], _mm(hist[:, k * 128:(k + 1) * 128]),
                             _mm(fct_s[0:H, 0:K]), start=True, stop=False,
                             skip_group_check=True)
            nc.tensor.matmul(pe[:], _mm(hist[:, TOK + k * 128:TOK + (k + 1) * 128]),
                             _mm(fct_s[0:H, K:2 * K]), start=False, stop=True,
                             skip_group_check=True)
            nc.vector.tensor_tensor(out=emsb[:, k * K:(k + 1) * K], in0=pe[:],
                                    in1=fcb_s[:], op=ALU.add)
            # e_scr[b*T*9 + (k*8+tt)*9 + j] <- emsb[p=(tt,b), k*9+j]
            nc.sync.dma_start(
                out=_ap(e_scr[:], k * TPT * K, [[K, TPT], [T * K, BL], [1, K]]),
                in_=emsb[:, k * K:(k + 1) * K])

        with tc.tile_pool(name="gpsum", bufs=2, space="PSUM") as gpp, \
             tc.tile_pool(name="fcp", bufs=2, space="PSUM") as fcp, \
             tc.tile_pool(name="lst", bufs=2) as lsp:
            g_cur = gpp.tile([128, GW], F32, tag="g")
            for slot in range(8):
                emit_xp(g_cur, 0, slot)
            for ch in range(NCH):
                g_nxt = None
                if ch + 1 < NCH:
                    g_nxt = gpp.tile([128, GW], F32, tag="g")
                for sl in range(TPC):
                    if g_nxt is not None and sl < 8:
                        emit_xp(g_nxt, ch + 1, sl)
                    emit_step(ch * TPC + sl, g_cur[:], lsp)
                if ch >= NCH // 2:
                    emit_fc(NCH - 1 - ch, fcp)
                    emit_fc(ch, fcp)
                g_cur = g_nxt

        wem = NTILE * K          # emsb row width

        # ---- phase 4: estream from e_scr ----
        # leg 2: e_scr -> estr[p=(blk*16+b), u*9+j]
        nc.sync.dma_start(
            out=estr[:],
            in_=_ap(e_scr[:], 0, [[ESW, NBLK], [T * K, BL], [1, ESW]]))
        nc.scalar.activation(out=estr[:], in_=estr[:], func=ACTF.Exp)

        # ---- phase 5: gold-path score (num) ----
        with tc.tile_pool(name="nump", bufs=3) as npool:
            kb = 0
            while kb < NTILE:
                wdt = min(8, NTILE - kb)
                oh = npool.tile([128, 8 * K], F32, tag="oh")
                ohn = npool.tile([128, 8 * K], F32, tag="ohn")
                emu = npool.tile([128, 8 * K], F32, tag="emu")
                p1 = npool.tile([128, 8 * K * K], F32, tag="p1")
                p2 = npool.tile([128, 8 * K * K], F32, tag="p2")
                iota_b = _ap(iot_s[:], 0, [[K, 128], [0, wdt], [1, K]])
                nc.vector.tensor_tensor(
                    out=_ap(oh[:], 0, [[8 * K, 128], [K, wdt], [1, K]]), in0=iota_b,
                    in1=_ap(tga_s[:], kb, [[NTILE, 128], [1, wdt], [0, K]]), op=ALU.is_equal)
                nc.vector.tensor_tensor(
                    out=_ap(ohn[:], 0, [[8 * K, 128], [K, wdt], [1, K]]), in0=iota_b,
                    in1=_ap(tgb_s[:], kb, [[NTILE, 128], [1, wdt], [0, K]]), op=ALU.is_equal)
                nc.vector.tensor_tensor(
                    out=_ap(emu[:], 0, [[8 * K, 128], [1, wdt * K]]),
                    in0=_ap(emsb[:], kb * K, [[wem, 128], [1, wdt * K]]),
                    in1=_ap(oh[:], 0, [[8 * K, 128], [1, wdt * K]]), op=ALU.mult)
                nc.vector.reduce_sum(
                    out=emcol[:, kb:kb + wdt],
                    in_=_ap(emu[:], 0, [[8 * K, 128], [K, wdt], [1, K]]), axis=AXL.X)
                nc.vector.tensor_tensor(
                    out=_ap(p1[:], 0, [[8 * K * K, 128], [K * K, wdt], [K, K], [1, K]]),
                    in0=_ap(oh[:], 0, [[8 * K, 128], [K, wdt], [1, K], [0, K]]),
                    in1=_ap(ohn[:], 0, [[8 * K, 128], [K, wdt], [0, K], [1, K]]),
                    op=ALU.mult)
                nc.vector.tensor_tensor(
                    out=_ap(p2[:], 0, [[8 * K * K, 128], [1, wdt * K * K]]),
                    in0=_ap(p1[:], 0, [[8 * K * K, 128], [1, wdt * K * K]]),
                    in1=_ap(t81_s[:], 0, [[K * K, 128], [0, wdt], [1, K * K]]), op=ALU.mult)
                nc.vector.reduce_sum(
                    out=trcol[:, kb:kb + wdt],
                    in_=_ap(p2[:], 0, [[8 * K * K, 128], [K * K, wdt], [K, K], [1, K]]),
                    axis=AXL.XY)
                kb += wdt

            sc_a = npool.tile([128, 1], F32, tag="oh")
            sc_b = npool.tile([128, 1], F32, tag="ohn")
            nc.vector.reduce_sum(out=sc_a[:], in_=emcol[:], axis=AXL.X)
            nc.vector.reduce_sum(out=sc_b[:], in_=trcol[:], axis=AXL.X)
            nc.vector.tensor_tensor(out=sc_a[:], in0=sc_a[:], in1=sc_b[:], op=ALU.add)
            # [128,1] -> [16,8] partition fold (p = r*16+b), via DRAM bounce
            s_scr = scr.tile([128, 1], F32)
            nc.sync.dma_start(out=s_scr[:], in_=sc_a[:])
            sc2 = npool.tile([BL, 8], F32, tag="emu")
            nc.sync.dma_start(
                out=_ap(sc2[:], 0, [[8, BL], [1, 8]]),
                in_=_ap(s_scr[:], 0, [[1, BL], [16, 8]]))
            num_t = pers.tile([BL, 1], F32)
            nc.vector.reduce_sum(out=num_t[:], in_=sc2[:], axis=AXL.X)
            # + start[tag0] + end[tagL]
            oh0 = npool.tile([BL, K], F32, tag="oh")
            m0 = npool.tile([BL, K], F32, tag="ohn")
            v0 = npool.tile([BL, 1], F32, tag="p1")
            for tgx, rep in ((tg0_s, srp_s[0:BL, :]), (tgL_s, erp_s[0:BL, :])):
                nc.vector.tensor_tensor(out=oh0[:], in0=iot_s[0:BL, :],
                                        in1=_ap(tgx[:], 0, [[1, BL], [0, K]]),
                                        op=ALU.is_equal)
                nc.vector.tensor_tensor(out=m0[:], in0=oh0[:], in1=rep, op=ALU.mult)
                nc.vector.reduce_sum(out=v0[:], in_=m0[:], axis=AXL.X)
                nc.vector.tensor_tensor(out=num_t[:], in0=num_t[:], in1=v0[:], op=ALU.add)

        # ---- phase 6: CRF partition function via product tree ----
        # G_t[i,j] = exp(trans[i,j]) * exp(e_t[j]); per partition p=(blk,b) a
        # row of 64 consecutive-t matrices.  6 levels of pairwise products.
        KK = K * K
        K3 = K * K * K
        Gt = pers.tile([128, UB * KK], F32)       # 64*81 = 5184
        Sb = pers.tile([128, 31], F32)            # norm slots, levels 2..6
        nc.vector.tensor_tensor(
            out=_ap(Gt[:], 0, [[UB * KK, 128], [KK, UB], [K, K], [1, K]]),
            in0=_ap(estr[:], 0, [[ESW, 128], [K, UB], [0, K], [1, K]]),
            in1=_ap(tex_s[:], 0, [[KK, 128], [0, UB], [K, K], [1, K]]),
            op=ALU.mult)
        # block 0 slot u=0 := I (t=0 has no transition; e_0 lives in a0)
        nc.vector.tensor_copy(out=Gt[0:BL, 0:KK], in_=i81_s[:])

        with tc.tile_pool(name="trp", bufs=3) as trp:
            cur = Gt
            soff = 0
            for lvl in range(1, 7):
                n = UB >> lvl                      # products this level
                nxt = pers.tile([128, n * KK], F32, name=f"lvl{lvl}")
                # sub-batches to bound tmp SBUF (<= 8 products per pass)
                step = min(n, 8)
                for s0 in range(0, n, step):
                    ns = min(step, n - s0)
                    tmp = trp.tile([128, 8 * K3], F32, tag="tmp")
                    # tmp[p, s,i,j,k] = A[p, s,i,k] * B[p, s,k,j]
                    # (HW ISA caps compute APs at 3 free dims -> unroll i)
                    cw = cur.shape[1]
                    for i in range(K):
                        eng = nc.vector if i < 6 else nc.gpsimd
                        eng.tensor_tensor(
                            out=_ap(tmp[:], i * KK,
                                    [[8 * K3, 128], [K3, ns], [K, K], [1, K]]),
                            in0=_ap(cur[:], s0 * 2 * KK + i * K,
                                    [[cw, 128], [2 * KK, ns], [0, K], [1, K]]),
                            in1=_ap(cur[:], s0 * 2 * KK + KK,
                                    [[cw, 128], [2 * KK, ns], [1, K], [K, K]]),
                            op=ALU.mult)
                    # nxt[p, s,i,j] = sum_k tmp[p, s,i,j,k]
                    nc.vector.reduce_sum(
                        out=_ap(nxt[:], s0 * KK,
                                [[n * KK, 128], [K, ns * K], [1, K]]),
                        in_=_ap(tmp[:], 0,
                                [[8 * K3, 128], [KK, ns * K], [K, K], [1, K]]),
                        axis=AXL.X)
                if lvl >= 2:
                    # sum-normalize each product; record norms for ln-fixup
                    rS = trp.tile([128, 16], F32, tag="rS")
                    nc.vector.reduce_sum(
                        out=Sb[:, soff:soff + n],
                        in_=_ap(nxt[:], 0, [[n * KK, 128], [KK, n], [1, KK]]),
                        axis=AXL.X)
                    nc.vector.reciprocal(out=rS[:, 0:n], in_=Sb[:, soff:soff + n])
                    nc.vector.tensor_tensor(
                        out=nxt[:], in0=nxt[:],
                        in1=_ap(rS[:], 0, [[16, 128], [1, n], [0, KK]]),
                        op=ALU.mult)
                    soff += n
                cur = nxt
            assert soff == 31

            # ---- combine: v = a0; for blk: v = v @ P_blk; Z = <v, q> ----
            p_scr = scr.tile([128, KK], F32)
            l_scr = scr.tile([128, 1], F32)
            nc.sync.dma_start(out=p_scr[:], in_=cur[:])
            pb = pers.tile([BL, NBLK * KK], F32)   # [b, blk*81 + i*9 + j]
            nc.sync.dma_start(
                out=pb[:],
                in_=_ap(p_scr[:], 0, [[KK, BL], [BL * KK, NBLK], [1, KK]]))
            # ln-norm fold: Ls[p] = sum ln Sb -> fold 8 partitions per b
            nc.scalar.activation(out=Sb[:], in_=Sb[:], func=ACTF.Ln)
            Ls = trp.tile([128, 1], F32, tag="rS")
            nc.vector.reduce_sum(out=Ls[:], in_=Sb[:], axis=AXL.X)
            nc.sync.dma_start(out=l_scr[:], in_=Ls[:])
            lb8 = pers.tile([BL, NBLK], F32)
            nc.sync.dma_start(
                out=lb8[:], in_=_ap(l_scr[:], 0, [[1, BL], [BL, NBLK]]))
            lnb = pers.tile([BL, 1], F32)
            nc.vector.reduce_sum(out=lnb[:], in_=lb8[:], axis=AXL.X)

            v_t = pers.tile([BL, K], F32)
            nc.vector.tensor_tensor(out=v_t[:], in0=sxp_s[:],
                                    in1=estr[0:BL, 0:K], op=ALU.mult)
            tv = trp.tile([BL, KK], F32, tag="tmp")
            for blk in range(NBLK):
                # tv[b, j*9+i] = v[b,i] * P[b, blk, i*9+j]
                nc.vector.tensor_tensor(
                    out=_ap(tv[:], 0, [[KK, BL], [K, K], [1, K]]),
                    in0=_ap(v_t[:], 0, [[K, BL], [0, K], [1, K]]),
                    in1=_ap(pb[:], blk * KK, [[NBLK * KK, BL], [1, K], [K, K]]),
                    op=ALU.mult)
                nc.vector.reduce_sum(
                    out=v_t[:], in_=_ap(tv[:], 0, [[KK, BL], [K, K], [1, K]]),
                    axis=AXL.X)
            # Z = <v, q>
            nc.vector.tensor_tensor(out=tv[:, 0:K], in0=v_t[:], in1=exq_s[:],
                                    op=ALU.mult)
            zt = pers.tile([BL, 1], F32)
            nc.vector.reduce_sum(out=zt[:], in_=tv[:, 0:K], axis=AXL.X)
            nc.scalar.activation(out=zt[:], in_=zt[:], func=ACTF.Ln)
            llh_t = pers.tile([BL, 1], F32)
            nc.vector.tensor_tensor(out=llh_t[:], in0=num_t[:], in1=zt[:],
                                    op=ALU.subtract)
            nc.vector.tensor_tensor(out=llh_t[:], in0=llh_t[:], in1=lnb[:],
                                    op=ALU.subtract)
            nc.sync.dma_start(out=llh_d[:], in_=llh_t[:])

    nc.compile()
    return nc


# ---------------- host side ----------------

def _prep_consts(T, wbf, hbf, embedding, W_ih_f, W_hh_f, b_f, W_ih_b, W_hh_b, b_b,
                 fc_W, fc_b, start_trans, end_trans, transitions):
    import ml_dtypes
    wdt = ml_dtypes.bfloat16 if wbf else np.float32
    hdt = ml_dtypes.bfloat16 if hbf else np.float32
    TOK = T * BL
    HB = 8 * H

    wih = np.zeros((E + 1, HB), np.float32)
    whh = np.zeros((H, HB), np.float32)
    for d, (Wi, Wh, bb) in enumerate(((W_ih_f, W_hh_f, b_f), (W_ih_b, W_hh_b, b_b))):
        for g in range(G):
            scale = 2.0 if g == 2 else 1.0  # tanh gate: tanh(x)=2*sig(2x)-1
            blk = slice((d * G + g) * H, (d * G + g + 1) * H)
            wih[0:E, blk] = scale * np.asarray(Wi)[g * H:(g + 1) * H, :].T
            wih[E, blk] = scale * np.asarray(bb)[g * H:(g + 1) * H]
            whh[:, blk] = scale * np.asarray(Wh)[g * H:(g + 1) * H, :].T

    fct = np.zeros((H, 2 * K), np.float32)
    fct[:, 0:K] = np.asarray(fc_W)[:, 0:H].T
    fct[:, K:2 * K] = np.asarray(fc_W)[:, H:2 * H].T

    tr = np.asarray(transitions, np.float32)
    consts = {
        "emb": np.asarray(embedding, np.float32),
        "wih": wih.astype(wdt),
        "whh": whh.astype(wdt),
        "fct": fct.astype(wdt),
        "fcb": np.tile(np.asarray(fc_b, np.float32)[None, :], (128, 1)),
        "iot": np.tile(np.arange(K, dtype=np.float32)[None, :], (128, 1)),
        "t81": np.tile(tr.reshape(1, K * K), (128, 1)),
        "tex": np.tile(np.exp(tr).reshape(1, K * K), (128, 1)),
        "i81": np.tile(np.eye(K, dtype=np.float32).reshape(1, K * K), (BL, 1)),
        "sxp": np.tile(np.exp(np.asarray(start_trans, np.float32))[None, :], (BL, 1)),
        "exq": np.tile(np.exp(np.asarray(end_trans, np.float32))[None, :], (BL, 1)),
        "srp": np.tile(np.asarray(start_trans, np.float32)[None, :], (BL, 1)),
        "erp": np.tile(np.asarray(end_trans, np.float32)[None, :], (BL, 1)),
        "one": np.ones((1, TOK), hdt),
    }
    return consts


def _core_inputs(T, consts, xl, tl):
    TOK = T * BL
    NTILE = TOK // 128
    ko = _korder(NTILE)
    xt = np.ascontiguousarray(xl.T).reshape(NTILE, 128)     # tile-major tokens
    idx = np.ascontiguousarray(xt[ko].T).astype(np.int32)   # [128, NTILE] p-major
    tga = np.ascontiguousarray(
        np.asarray(tl).T.reshape(NTILE, 128).T).astype(np.float32)
    tshift = np.concatenate([tl[:, 1:], np.full((BL, 1), K, tl.dtype)], axis=1)
    tgb = np.ascontiguousarray(
        np.asarray(tshift).T.reshape(NTILE, 128).T).astype(np.float32)
    m = dict(consts)
    m.update({
        "idx": idx, "tga": tga, "tgb": tgb,
        "tg0": tl[:, 0:1].astype(np.float32),
        "tgL": tl[:, T - 1:T].astype(np.float32),
    })
    return m


def run_cores(T, V, inputs_full, n_cores=8, wbf=False, hbf=False, trace=False):
    """Build + run on n_cores; returns np.float32 scalar loss (and exec ns if trace)."""
    from concourse.bass_utils import run_bass_kernel_spmd
    x = np.asarray(inputs_full["x"])
    tags = np.asarray(inputs_full["tags"])
    consts = _prep_consts(
        T, wbf, hbf, inputs_full["embedding"],
        inputs_full["W_ih_f"], inputs_full["W_hh_f"], inputs_full["b_f"],
        inputs_full["W_ih_b"], inputs_full["W_hh_b"], inputs_full["b_b"],
        inputs_full["fc_W"], inputs_full["fc_b"],
        inputs_full["start_trans"], inputs_full["end_trans"], inputs_full["transitions"])
    nc = build_program(T=T, V=V, wbf=wbf, hbf=hbf)
    in_maps = [
        _core_inputs(T, consts, x[c * BL:(c + 1) * BL], tags[c * BL:(c + 1) * BL])
        for c in range(n_cores)
    ]
    res = run_bass_kernel_spmd(nc, in_maps, list(range(n_cores)), trace=trace)
    llh = np.stack([r["llh"] for r in res.results])
    ntotal = n_cores * BL
    loss = np.float32(-(llh.sum() / ntotal))
    if trace:
        return loss, res.exec_time_ns, getattr(res, "instructions_and_trace", None)
    return loss


def kernel(x, tags, mask, embedding, W_ih_f, W_hh_f, b_f, W_ih_b, W_hh_b, b_b,
           fc_W, fc_b, start_trans, end_trans, transitions):
    # mask is all ones per problem spec; not applied.
    return run_cores(512, 30000, wbf=True, hbf=True, inputs_full={
        "x": x, "tags": tags, "embedding": embedding,
        "W_ih_f": W_ih_f, "W_hh_f": W_hh_f, "b_f": b_f,
        "W_ih_b": W_ih_b, "W_hh_b": W_hh_b, "b_b": b_b,
        "fc_W": fc_W, "fc_b": fc_b, "start_trans": start_trans,
        "end_trans": end_trans, "transitions": transitions,
    })
